# revision 16
# baseline (speedup 1.0000x reference)
"""BufferAttend1d on 8 NeuronCores (TRN2, Bass/Tile).

Sharding: data-parallel over batch (4 batches x 2 cores), sequence-parallel
over Q within a batch (2048 queries/core), full K/V buffer per core.

Per-core device layout ("layout B"): the softmax/contraction k-dim stays on
SBUF partitions throughout:
  queryT [64, 2048], keysT [64, 4096]       (h on partitions)
  vals_ext [4096, 65]                       (k on partitions; col 64 = ones)
  logitsT/eTm tiles [128k, 1024q]           (k on partitions)
Host pre-transposes x, buffer, mask and weights so all device matmuls have
their contraction dim on partitions (no large on-chip transposes), and sends
x/buffer/weights as fp16 (halves DMA, enables full-rate warm matmuls - f32r
keeps the PE HAM clock-gate cold and runs at half clock).
The ones-column in vals makes the PV matmul also produce the softmax
denominator (row 64 of readT); the final 128x65 PE-transpose puts the
denominator on the free axis for a per-partition tensor_scalar divide.
Masking: eTm = exp(logits/8) * mask, multiplied as u8 {0,1}; exact zeros match
the reference's exp(-1024 - max) == 0 in fp32.
"""

import numpy as np
from contextlib import ExitStack

# Per-core shard shapes (hardcoded per problem spec)
B = 4
QF = 4096          # full queries per batch
Q = 2048           # queries per core
K = 4096           # keys
D = 512            # model dim
H = 64             # key/value head dim
QL = 1024          # q block for the logits/exp/mask/PV stage
NQB = Q // QL      # 2
NKT = K // 128     # 32 k-tiles
NDC = D // 128     # 4 d-chunks
SCALE = 1.0 / 8.0  # 1/sqrt(64)

_CACHE = {}
_ONESC = np.ones((128, K // 128, 1), np.float16)
import ml_dtypes as _mld
_F8 = _mld.float8_e4m3
_NEGID8 = (np.eye(128, dtype=np.float32) * -240.0).astype(_F8)


def _build_program():
    import concourse.mybir as mybir
    import concourse.tile as tile
    from concourse import bacc
    from concourse.masks import make_identity

    f32 = mybir.dt.float32
    f16 = mybir.dt.float16
    f8 = mybir.dt.float8e4
    u8 = mybir.dt.uint8
    AF = mybir.ActivationFunctionType
    OP = mybir.AluOpType

    nc = bacc.Bacc("TRN2", target_bir_lowering=False, debug=False, num_devices=8)

    xT = nc.dram_tensor("xT", [D, Q], f16, kind="ExternalInput").ap()
    bufT = nc.dram_tensor("bufT", [D, K], f16, kind="ExternalInput").ap()
    maskT = nc.dram_tensor("maskT", [K, Q], f8, kind="ExternalInput").ap()
    negid8 = nc.dram_tensor("negid8", [128, 128], f8, kind="ExternalInput").ap()
    wqT = nc.dram_tensor("wqT", [D, H], f16, kind="ExternalInput").ap()
    wkT = nc.dram_tensor("wkT", [D, H], f16, kind="ExternalInput").ap()
    wvT = nc.dram_tensor("wvT", [D, H], f16, kind="ExternalInput").ap()
    bq2 = nc.dram_tensor("bq2", [H, 1], f32, kind="ExternalInput").ap()
    bk2 = nc.dram_tensor("bk2", [H, 1], f32, kind="ExternalInput").ap()
    bv2 = nc.dram_tensor("bv2", [H, 1], f32, kind="ExternalInput").ap()
    onesc = nc.dram_tensor("onesc", [128, NKT, 1], f16, kind="ExternalInput").ap()
    out = nc.dram_tensor("out", [Q, H], f32, kind="ExternalOutput").ap()

    xT_c = xT.rearrange("(c p) q -> c p q", p=128)       # [NDC, 128, Q]
    bufT_c = bufT.rearrange("(c p) k -> c p k", p=128)   # [NDC, 128, K]
    maskT_v = maskT.rearrange("(t p) q -> p t q", p=128)  # [128, NKT, Q]
    wqT_c = wqT.rearrange("(c p) h -> p c h", p=128)     # [128, NDC, H]
    wkT_c = wkT.rearrange("(c p) h -> p c h", p=128)
    wvT_c = wvT.rearrange("(c p) h -> p c h", p=128)

    with tile.TileContext(nc) as tc, ExitStack() as ctx:
        const = ctx.enter_context(tc.tile_pool(name="const", bufs=1))
        persist = ctx.enter_context(tc.tile_pool(name="persist", bufs=1))

        # constants
        wq_sb = const.tile([128, NDC, H], f16, tag="wq")
        wk_sb = const.tile([128, NDC, H], f16, tag="wk")
        wv_sb = const.tile([128, NDC, H], f16, tag="wv")
        nc.sync.dma_start(out=wq_sb, in_=wqT_c)
        nc.sync.dma_start(out=wk_sb, in_=wkT_c)
        nc.sync.dma_start(out=wv_sb, in_=wvT_c)
        bq_sb = const.tile([H, 1], f32, tag="bq")
        bk_sb = const.tile([H, 1], f32, tag="bk")
        bv_sb2 = const.tile([H, 1], f32, tag="bv2")
        nc.sync.dma_start(out=bq_sb, in_=bq2)
        nc.sync.dma_start(out=bk_sb, in_=bk2)
        nc.sync.dma_start(out=bv_sb2, in_=bv2)
        ident = const.tile([128, 128], f32, tag="ident")
        make_identity(nc, ident)
        ident16 = const.tile([128, 128], f16, tag="ident16")
        make_identity(nc, ident16)
        negid8_sb = const.tile([128, 128], f8, tag="negid8")
        nc.sync.dma_start(out=negid8_sb, in_=negid8)

        # persistent activations; queryT/keysT duplicated on partitions 64-127
        # so pairs of QK matmuls can run on distinct PE row-groups concurrently
        queryT = persist.tile([128, Q], f16, tag="queryT")
        keysT = persist.tile([128, K], f16, tag="keysT")
        vals = persist.tile([128, NKT, H + 1], f16, tag="vals")

        # ---- Phase 1: projections ----
        with tc.tile_pool(name="xin", bufs=NDC) as xin_pool, \
             tc.tile_pool(name="bin", bufs=NDC) as bin_pool, \
             tc.tile_pool(name="pp64", bufs=2, space="PSUM") as pp64, \
             tc.tile_pool(name="ppv", bufs=2, space="PSUM") as ppv:
            xt = []
            for dc in range(NDC):
                t = xin_pool.tile([128, Q], f16)
                nc.sync.dma_start(out=t, in_=xT_c[dc])
                xt.append(t)
            bt = []
            for dc in range(NDC):
                t = bin_pool.tile([128, K], f16)
                nc.sync.dma_start(out=t, in_=bufT_c[dc])
                bt.append(t)

            # queryT[h, q] = sum_d WqT[d, h] * xT[d, q]  (+ bq per-partition)
            for qb in range(Q // 512):
                ps = pp64.tile([H, 512], f32, tag="p64")
                qs = slice(qb * 512, (qb + 1) * 512)
                for dc in range(NDC):
                    nc.tensor.matmul(ps, lhsT=wq_sb[:, dc, :], rhs=xt[dc][:, qs],
                                     start=(dc == 0), stop=(dc == NDC - 1))
                nc.scalar.activation(out=queryT[0:H, qs], in_=ps, func=AF.Identity,
                                     bias=bq_sb, scale=1.0)
            # keysT[h, k]
            for kb in range(K // 512):
                ps = pp64.tile([H, 512], f32, tag="p64")
                ks = slice(kb * 512, (kb + 1) * 512)
                for dc in range(NDC):
                    nc.tensor.matmul(ps, lhsT=wk_sb[:, dc, :], rhs=bt[dc][:, ks],
                                     start=(dc == 0), stop=(dc == NDC - 1))
                nc.scalar.activation(out=keysT[0:H, ks], in_=ps, func=AF.Identity,
                                     bias=bk_sb, scale=1.0)
            # valsT[v, k] (fast N=512 matmuls), add bias via per-partition ACT,
            # then PE-transpose 64x128 tiles into vals[k, v] natural layout.
            valsT = persist.tile([H, K], f16, tag="valsT")
            for kb in range(K // 512):
                ps = pp64.tile([H, 512], f32, tag="p64")
                ks = slice(kb * 512, (kb + 1) * 512)
                for dc in range(NDC):
                    nc.tensor.matmul(ps, lhsT=wv_sb[:, dc, :], rhs=bt[dc][:, ks],
                                     start=(dc == 0), stop=(dc == NDC - 1))
                nc.scalar.activation(out=valsT[:, ks], in_=ps, func=AF.Identity,
                                     bias=bv_sb2, scale=1.0)
            for kt in range(NKT):
                pv16 = ppv.tile([128, H], f16, tag="pv")
                nc.tensor.transpose(pv16, valsT[:, kt * 128:(kt + 1) * 128],
                                    ident16[0:H, 0:H])
                nc.scalar.copy(out=vals[:, kt, 0:H], in_=pv16)
            nc.sync.dma_start(out=vals[:, :, H:H + 1], in_=onesc)
            nc.sync.dma_start(out=queryT[H:2 * H, :], in_=queryT[0:H, :])
            nc.sync.dma_start(out=keysT[H:2 * H, :], in_=keysT[0:H, :])

        # ---- Phase 2: attention over q-blocks of QL ----
        rts = []
        with tc.tile_pool(name="mask", bufs=2) as mask_pool, \
             tc.tile_pool(name="et", bufs=NKT + 2) as et_pool, \
             tc.tile_pool(name="rt", bufs=NQB) as rt_pool, \
             tc.tile_pool(name="psl", bufs=3, space="PSUM") as psl_pool, \
             tc.tile_pool(name="psr", bufs=1, space="PSUM") as psr_pool:
            for qb in range(NQB):
                qs = slice(qb * QL, (qb + 1) * QL)
                mslab = mask_pool.tile([128, NKT, QL], f8, tag="m")
                nc.sync.dma_start(out=mslab, in_=maskT_v[:, :, qs])
                ets = []
                for pair in range(NKT // 2):
                    kta, ktb = 2 * pair, 2 * pair + 1
                    pls = {kta: psl_pool.tile([128, QL], f32, tag="l", name=f"pl{qb}_{kta}"),
                           ktb: psl_pool.tile([128, QL], f32, tag="l", name=f"pl{qb}_{ktb}")}
                    for h in range(QL // 512):
                        hs = slice(h * 512, (h + 1) * 512)
                        qhs = slice(qb * QL + h * 512, qb * QL + (h + 1) * 512)
                        nc.tensor.matmul(pls[kta][:, hs],
                                         lhsT=keysT[0:H, kta * 128:(kta + 1) * 128],
                                         rhs=queryT[0:H, qhs],
                                         start=True, stop=False)
                        nc.tensor.matmul(pls[ktb][:, hs],
                                         lhsT=keysT[H:2 * H, ktb * 128:(ktb + 1) * 128],
                                         rhs=queryT[H:2 * H, qhs],
                                         start=True, stop=False)
                        # add -448 to masked-out logits: exp underflows to 0
                        nc.tensor.matmul(pls[kta][:, hs], lhsT=negid8_sb,
                                         rhs=mslab[:, kta, hs], start=False, stop=True)
                        nc.tensor.matmul(pls[ktb][:, hs], lhsT=negid8_sb,
                                         rhs=mslab[:, ktb, hs], start=False, stop=True)
                    for kt in (kta, ktb):
                        et = et_pool.tile([128, QL], f16, tag="e")
                        nc.scalar.activation(out=et, in_=pls[kt], func=AF.Exp,
                                             scale=SCALE)
                        ets.append(et)
                pr = psr_pool.tile([H + 1, QL], f32, tag="r")
                for kt in range(NKT):
                    for h in range(QL // 512):
                        hs = slice(h * 512, (h + 1) * 512)
                        nc.tensor.matmul(pr[:, hs], lhsT=vals[:, kt, :],
                                         rhs=ets[kt][:, hs],
                                         start=(kt == 0), stop=(kt == NKT - 1))
                rt = rt_pool.tile([H + 1, QL], f32, tag="rt")
                nc.vector.tensor_copy(out=rt, in_=pr)
                rts.append(rt)

        # ---- Phase 3: transpose readT, divide by denominator, store ----
        with tc.tile_pool(name="ot", bufs=3) as ot_pool, \
             tc.tile_pool(name="rec", bufs=3) as rec_pool, \
             tc.tile_pool(name="pst", bufs=3, space="PSUM") as pst_pool:
            for qb in range(NQB):
                rt = rts[qb]
                for j in range(QL // 128):
                    pt = pst_pool.tile([128, H + 1], f32, tag="t")
                    nc.tensor.transpose(pt, rt[:, j * 128:(j + 1) * 128],
                                        ident[0:H + 1, 0:H + 1])
                    rec = rec_pool.tile([128, 1], f32, tag="rc")
                    nc.vector.reciprocal(out=rec, in_=pt[:, H:H + 1])
                    ot = ot_pool.tile([128, H], f32, tag="o")
                    nc.vector.tensor_scalar(out=ot, in0=pt[:, 0:H], scalar1=rec,
                                            scalar2=None, op0=OP.mult)
                    r0 = qb * QL + j * 128
                    nc.sync.dma_start(out=out[r0:r0 + 128, :], in_=ot)

    nc.compile()
    return nc


def _get_nc():
    if "nc" not in _CACHE:
        _CACHE["nc"] = _build_program()
    return _CACHE["nc"]


def _make_in_maps(x, buffer, mask, Wq, bq, Wk, bk, Wv, bv):
    x = np.asarray(x, dtype=np.float32)
    buffer = np.asarray(buffer, dtype=np.float32)
    mask = np.asarray(mask)
    wqT = np.ascontiguousarray(np.asarray(Wq, np.float32).T).astype(np.float16)
    wkT = np.ascontiguousarray(np.asarray(Wk, np.float32).T).astype(np.float16)
    wvT = np.ascontiguousarray(np.asarray(Wv, np.float32).T).astype(np.float16)
    bq2 = np.ascontiguousarray(np.asarray(bq, np.float32).reshape(H, 1))
    bk2 = np.ascontiguousarray(np.asarray(bk, np.float32).reshape(H, 1))
    bv2 = np.ascontiguousarray(np.asarray(bv, np.float32).reshape(H, 1))
    in_maps = []
    for c in range(8):
        b, half = divmod(c, 2)
        qs = slice(half * Q, (half + 1) * Q)
        maskT_c = np.ascontiguousarray(~(mask[b, qs].astype(bool)).T)
        maskT_c = maskT_c.astype(np.float32).astype(_F8)
        in_maps.append({
            "xT": np.ascontiguousarray(x[b, qs].T).astype(np.float16),
            "bufT": np.ascontiguousarray(buffer[b].T).astype(np.float16),
            "maskT": maskT_c,
            "wqT": wqT, "wkT": wkT, "wvT": wvT,
            "bq2": bq2, "bk2": bk2, "bv2": bv2,
            "onesc": _ONESC, "negid8": _NEGID8,
        })
    return in_maps


def _make_sharded(nc, in_maps):
    """8-device dispatch mirroring bass2jax.run_bass_via_pjrt's multicore path,
    but with inputs device_put + blocked before execute (avoids a first-run
    upload/execute race seen on high-numbered cores)."""
    import jax
    from jax.sharding import Mesh, PartitionSpec, NamedSharding
    from jax.experimental.shard_map import shard_map
    from concourse import bass2jax, mybir
    from concourse.bass2jax import _bass_exec_p, partition_id_tensor

    bass2jax.install_neuronx_cc_hook()
    n_cores = len(in_maps)
    partition_name = nc.partition_id_tensor.name if nc.partition_id_tensor else None
    in_names, out_names, out_avals, zero_shapes = [], [], [], []
    for alloc in nc.m.functions[0].allocations:
        if not isinstance(alloc, mybir.MemoryLocationSet):
            continue
        name = alloc.memorylocations[0].name
        if alloc.kind == "ExternalInput":
            if name != partition_name:
                in_names.append(name)
        elif alloc.kind == "ExternalOutput":
            shape = tuple(alloc.tensor_shape)
            dtype = mybir.dt.np(alloc.dtype)
            out_names.append(name)
            out_avals.append(jax.core.ShapedArray(shape, dtype))
            zero_shapes.append((shape, dtype))
    n_params = len(in_names)
    n_outs = len(out_avals)
    in_names_all = in_names + out_names + ([partition_name] if partition_name else [])
    donate = tuple(range(n_params, n_params + n_outs))

    def _body(*args):
        operands = list(args)
        if partition_name is not None:
            operands.append(partition_id_tensor())
        outs = _bass_exec_p.bind(
            *operands, out_avals=tuple(out_avals), in_names=tuple(in_names_all),
            out_names=tuple(out_names), lowering_input_output_aliases=(),
            sim_require_finite=True, sim_require_nnan=True, nc=nc)
        return tuple(outs)

    devices = jax.devices()[:n_cores]
    mesh = Mesh(np.asarray(devices), ("core",))
    sharded = jax.jit(
        shard_map(_body, mesh=mesh,
                  in_specs=(PartitionSpec("core"),) * (n_params + n_outs),
                  out_specs=(PartitionSpec("core"),) * n_outs,
                  check_rep=False),
        donate_argnums=donate, keep_unused=True)
    sh = NamedSharding(mesh, PartitionSpec("core"))

    concat_in = [np.concatenate([np.asarray(in_maps[c][name]) for c in range(n_cores)],
                                axis=0) for name in in_names]
    dev_in = [jax.device_put(a, sh) for a in concat_in]
    for a in dev_in:
        a.block_until_ready()

    def run_once():
        zeros = [jax.device_put(np.zeros((n_cores * s[0],) + tuple(s[1:]), d), sh)
                 for s, d in zero_shapes]
        for z in zeros:
            z.block_until_ready()
        outs = sharded(*dev_in, *zeros)
        for o in outs:
            o.block_until_ready()
        return [{name: np.asarray(outs[i]).reshape(n_cores, *out_avals[i].shape)[c]
                 for i, name in enumerate(out_names)} for c in range(n_cores)]

    return run_once


def _gather(results):
    out = np.empty((B, QF, H), np.float32)
    for c in range(8):
        b, half = divmod(c, 2)
        out[b, half * Q:(half + 1) * Q] = results[c]["out"]
    return out


def _run(inputs, trace=False):
    nc = _get_nc()
    in_maps = _make_in_maps(**inputs)
    if trace:
        # warm-up execute first (first run after load can race input upload),
        # then a traced execute via run_bass_kernel_spmd for the NTFF profile.
        from concourse.bass_utils import run_bass_kernel_spmd
        run_bass_kernel_spmd(nc, in_maps, list(range(8)), trace=False)
        res = run_bass_kernel_spmd(nc, in_maps, list(range(8)), trace=True)
        return _gather(res.results), res
    run_once = _make_sharded(nc, in_maps)
    run_once()          # warm-up: discard first execution
    results = run_once()
    return _gather(results), None


def kernel(**inputs):
    out, _ = _run(inputs, trace=False)
    return out


# revision 19
# speedup vs baseline: 1.0606x; 1.0606x over previous
"""BufferAttend1d on 8 NeuronCores (TRN2, Bass/Tile).

Sharding: data-parallel over batch (4 batches x 2 cores), sequence-parallel
over Q within a batch (2048 queries/core), full K/V buffer per core.

Per-core device layout ("layout B"): the softmax/contraction k-dim stays on
SBUF partitions throughout:
  queryT [64, 2048], keysT [64, 4096]       (h on partitions)
  vals_ext [4096, 65]                       (k on partitions; col 64 = ones)
  logitsT/eTm tiles [128k, 1024q]           (k on partitions)
Host pre-transposes x, buffer, mask and weights so all device matmuls have
their contraction dim on partitions (no large on-chip transposes), and sends
x/buffer/weights as fp16 (halves DMA, enables full-rate warm matmuls - f32r
keeps the PE HAM clock-gate cold and runs at half clock).
The ones-column in vals makes the PV matmul also produce the softmax
denominator (row 64 of readT); the final 128x65 PE-transpose puts the
denominator on the free axis for a per-partition tensor_scalar divide.
Masking: eTm = exp(logits/8) * mask, multiplied as u8 {0,1}; exact zeros match
the reference's exp(-1024 - max) == 0 in fp32.
"""

import numpy as np
from contextlib import ExitStack

# Per-core shard shapes (hardcoded per problem spec)
B = 4
QF = 4096          # full queries per batch
Q = 2048           # queries per core
K = 4096           # keys
D = 512            # model dim
H = 64             # key/value head dim
QL = 1024          # q block for the logits/exp/mask/PV stage
NQB = Q // QL      # 2
NKT = K // 128     # 32 k-tiles
NDC = D // 128     # 4 d-chunks
SCALE = 1.0 / 8.0  # 1/sqrt(64)

_CACHE = {}
_ONESC = np.ones((128, K // 128, 1), np.float16)
import ml_dtypes as _mld
_F8 = _mld.float8_e4m3
_NEGID8 = (np.eye(128, dtype=np.float32) * -240.0).astype(_F8)


def _build_program():
    import concourse.mybir as mybir
    import concourse.tile as tile
    from concourse import bacc
    from concourse.masks import make_identity

    f32 = mybir.dt.float32
    f16 = mybir.dt.float16
    f8 = mybir.dt.float8e4
    u8 = mybir.dt.uint8
    AF = mybir.ActivationFunctionType
    OP = mybir.AluOpType

    from concourse.tile import add_dep_helper

    nc = bacc.Bacc("TRN2", target_bir_lowering=False, debug=False, num_devices=8)

    pe_prev = [None]

    def pe_chain(inst):
        raw = inst.ins if hasattr(inst, "ins") else inst
        if pe_prev[0] is not None:
            add_dep_helper(raw, pe_prev[0], sync=False,
                           reason="pin PE stream order")
        pe_prev[0] = raw
        return inst

    xT = nc.dram_tensor("xT", [D, Q], f16, kind="ExternalInput").ap()
    bufT = nc.dram_tensor("bufT", [D, K], f16, kind="ExternalInput").ap()
    maskT = nc.dram_tensor("maskT", [K, Q], f8, kind="ExternalInput").ap()
    negid8 = nc.dram_tensor("negid8", [128, 128], f8, kind="ExternalInput").ap()
    wqT = nc.dram_tensor("wqT", [D, H], f16, kind="ExternalInput").ap()
    wkT = nc.dram_tensor("wkT", [D, H], f16, kind="ExternalInput").ap()
    wvT = nc.dram_tensor("wvT", [D, H], f16, kind="ExternalInput").ap()
    bq2 = nc.dram_tensor("bq2", [H, 1], f32, kind="ExternalInput").ap()
    bk2 = nc.dram_tensor("bk2", [H, 1], f32, kind="ExternalInput").ap()
    bv2 = nc.dram_tensor("bv2", [H, 1], f32, kind="ExternalInput").ap()
    onesc = nc.dram_tensor("onesc", [128, NKT, 1], f16, kind="ExternalInput").ap()
    out = nc.dram_tensor("out", [Q, H], f32, kind="ExternalOutput").ap()

    xT_c = xT.rearrange("(c p) q -> c p q", p=128)       # [NDC, 128, Q]
    bufT_c = bufT.rearrange("(c p) k -> c p k", p=128)   # [NDC, 128, K]
    maskT_v = maskT.rearrange("(t p) q -> p t q", p=128)  # [128, NKT, Q]
    wqT_c = wqT.rearrange("(c p) h -> p c h", p=128)     # [128, NDC, H]
    wkT_c = wkT.rearrange("(c p) h -> p c h", p=128)
    wvT_c = wvT.rearrange("(c p) h -> p c h", p=128)

    with tile.TileContext(nc) as tc, ExitStack() as ctx:
        const = ctx.enter_context(tc.tile_pool(name="const", bufs=1))
        persist = ctx.enter_context(tc.tile_pool(name="persist", bufs=1))

        # constants
        wq_sb = const.tile([128, NDC, H], f16, tag="wq")
        wk_sb = const.tile([128, NDC, H], f16, tag="wk")
        wv_sb = const.tile([128, NDC, H], f16, tag="wv")
        nc.sync.dma_start(out=wq_sb, in_=wqT_c)
        nc.sync.dma_start(out=wk_sb, in_=wkT_c)
        nc.sync.dma_start(out=wv_sb, in_=wvT_c)
        bq_sb = const.tile([H, 1], f32, tag="bq")
        bk_sb = const.tile([H, 1], f32, tag="bk")
        bv_sb2 = const.tile([H, 1], f32, tag="bv2")
        nc.sync.dma_start(out=bq_sb, in_=bq2)
        nc.sync.dma_start(out=bk_sb, in_=bk2)
        nc.sync.dma_start(out=bv_sb2, in_=bv2)
        ident = const.tile([128, 128], f32, tag="ident")
        make_identity(nc, ident)
        ident16 = const.tile([128, 128], f16, tag="ident16")
        make_identity(nc, ident16)
        negid8_sb = const.tile([128, 128], f8, tag="negid8")
        nc.sync.dma_start(out=negid8_sb, in_=negid8)

        # persistent activations; queryT/keysT duplicated on partitions 64-127
        # so pairs of QK matmuls can run on distinct PE row-groups concurrently
        queryT = persist.tile([128, Q], f16, tag="queryT")
        keysT = persist.tile([128, K], f16, tag="keysT")
        vals = persist.tile([128, NKT, H + 1], f16, tag="vals")

        # ---- Phase 1: projections ----
        with tc.tile_pool(name="xin", bufs=NDC) as xin_pool, \
             tc.tile_pool(name="bin", bufs=NDC) as bin_pool, \
             tc.tile_pool(name="pp64", bufs=2, space="PSUM") as pp64, \
             tc.tile_pool(name="ppv", bufs=2, space="PSUM") as ppv:
            xt = []
            for dc in range(NDC):
                t = xin_pool.tile([128, Q], f16)
                nc.sync.dma_start(out=t, in_=xT_c[dc])
                xt.append(t)
            bt = []
            for dc in range(NDC):
                t = bin_pool.tile([128, K], f16)
                nc.sync.dma_start(out=t, in_=bufT_c[dc])
                bt.append(t)

            # queryT[h, q] = sum_d WqT[d, h] * xT[d, q]  (+ bq per-partition)
            for qb in range(Q // 512):
                ps = pp64.tile([H, 512], f32, tag="p64")
                qs = slice(qb * 512, (qb + 1) * 512)
                for dc in range(NDC):
                    pe_chain(nc.tensor.matmul(ps, lhsT=wq_sb[:, dc, :],
                                              rhs=xt[dc][:, qs],
                                              start=(dc == 0), stop=(dc == NDC - 1)))
                nc.scalar.activation(out=queryT[0:H, qs], in_=ps, func=AF.Identity,
                                     bias=bq_sb, scale=1.0)
            # keysT[h, k]
            for kb in range(K // 512):
                ps = pp64.tile([H, 512], f32, tag="p64")
                ks = slice(kb * 512, (kb + 1) * 512)
                for dc in range(NDC):
                    pe_chain(nc.tensor.matmul(ps, lhsT=wk_sb[:, dc, :],
                                              rhs=bt[dc][:, ks],
                                              start=(dc == 0), stop=(dc == NDC - 1)))
                nc.scalar.activation(out=keysT[0:H, ks], in_=ps, func=AF.Identity,
                                     bias=bk_sb, scale=1.0)
            # valsT[v, k] (fast N=512 matmuls), add bias via per-partition ACT,
            # then PE-transpose 64x128 tiles into vals[k, v] natural layout.
            valsT = persist.tile([H, K], f16, tag="valsT")
            for kb in range(K // 512):
                ps = pp64.tile([H, 512], f32, tag="p64")
                ks = slice(kb * 512, (kb + 1) * 512)
                for dc in range(NDC):
                    pe_chain(nc.tensor.matmul(ps, lhsT=wv_sb[:, dc, :],
                                              rhs=bt[dc][:, ks],
                                              start=(dc == 0), stop=(dc == NDC - 1)))
                nc.scalar.activation(out=valsT[:, ks], in_=ps, func=AF.Identity,
                                     bias=bv_sb2, scale=1.0)
            for kt in range(NKT):
                pv16 = ppv.tile([128, H], f16, tag="pv")
                pe_chain(nc.tensor.transpose(pv16, valsT[:, kt * 128:(kt + 1) * 128],
                                             ident16[0:H, 0:H]))
                nc.scalar.copy(out=vals[:, kt, 0:H], in_=pv16)
            nc.sync.dma_start(out=vals[:, :, H:H + 1], in_=onesc)
            nc.sync.dma_start(out=queryT[H:2 * H, :], in_=queryT[0:H, :])
            nc.sync.dma_start(out=keysT[H:2 * H, :], in_=keysT[0:H, :])

        # ---- Phase 2: attention over q-blocks of QL ----
        rts = []
        with tc.tile_pool(name="mask", bufs=2) as mask_pool, \
             tc.tile_pool(name="et", bufs=NKT + 2) as et_pool, \
             tc.tile_pool(name="rt", bufs=NQB) as rt_pool, \
             tc.tile_pool(name="psl", bufs=3, space="PSUM") as psl_pool, \
             tc.tile_pool(name="psr", bufs=1, space="PSUM") as psr_pool:
            for qb in range(NQB):
                qs = slice(qb * QL, (qb + 1) * QL)
                mslab = mask_pool.tile([128, NKT, QL], f8, tag="m")
                nc.sync.dma_start(out=mslab, in_=maskT_v[:, :, qs])
                ets = []
                pr = psr_pool.tile([H + 1, QL], f32, tag="r")

                def emit_pv(kt):
                    for h in range(QL // 512):
                        hs = slice(h * 512, (h + 1) * 512)
                        pe_chain(nc.tensor.matmul(
                            pr[:, hs], lhsT=vals[:, kt, :], rhs=ets[kt][:, hs],
                            start=(kt == 0), stop=(kt == NKT - 1)))

                PV_LAG = 2  # pairs
                for pair in range(NKT // 2):
                    kta, ktb = 2 * pair, 2 * pair + 1
                    pls = {kta: psl_pool.tile([128, QL], f32, tag="l", name=f"pl{qb}_{kta}"),
                           ktb: psl_pool.tile([128, QL], f32, tag="l", name=f"pl{qb}_{ktb}")}
                    for h in range(QL // 512):
                        hs = slice(h * 512, (h + 1) * 512)
                        qhs = slice(qb * QL + h * 512, qb * QL + (h + 1) * 512)
                        # adjacent pair on PE row-groups 0-63 / 64-127
                        pe_chain(nc.tensor.matmul(
                            pls[kta][:, hs],
                            lhsT=keysT[0:H, kta * 128:(kta + 1) * 128],
                            rhs=queryT[0:H, qhs], start=True, stop=False))
                        pe_chain(nc.tensor.matmul(
                            pls[ktb][:, hs],
                            lhsT=keysT[H:2 * H, ktb * 128:(ktb + 1) * 128],
                            rhs=queryT[H:2 * H, qhs], start=True, stop=False))
                        # add -240 to masked-out logits: exp underflows to 0
                        pe_chain(nc.tensor.matmul(
                            pls[kta][:, hs], lhsT=negid8_sb,
                            rhs=mslab[:, kta, hs], start=False, stop=True))
                        pe_chain(nc.tensor.matmul(
                            pls[ktb][:, hs], lhsT=negid8_sb,
                            rhs=mslab[:, ktb, hs], start=False, stop=True))
                    for kt in (kta, ktb):
                        et = et_pool.tile([128, QL], f16, tag="e")
                        nc.scalar.activation(out=et, in_=pls[kt], func=AF.Exp,
                                             scale=SCALE)
                        ets.append(et)
                    if pair >= PV_LAG:
                        for kt in (2 * (pair - PV_LAG), 2 * (pair - PV_LAG) + 1):
                            emit_pv(kt)
                for pair in range(NKT // 2 - PV_LAG, NKT // 2):
                    for kt in (2 * pair, 2 * pair + 1):
                        emit_pv(kt)
                rt = rt_pool.tile([H + 1, QL], f32, tag="rt")
                nc.vector.tensor_copy(out=rt, in_=pr)
                rts.append(rt)

        # ---- Phase 3: transpose readT, divide by denominator, store ----
        with tc.tile_pool(name="ot", bufs=3) as ot_pool, \
             tc.tile_pool(name="rec", bufs=3) as rec_pool, \
             tc.tile_pool(name="pst", bufs=3, space="PSUM") as pst_pool:
            for qb in range(NQB):
                rt = rts[qb]
                for j in range(QL // 128):
                    pt = pst_pool.tile([128, H + 1], f32, tag="t")
                    pe_chain(nc.tensor.transpose(pt, rt[:, j * 128:(j + 1) * 128],
                                                 ident[0:H + 1, 0:H + 1]))
                    rec = rec_pool.tile([128, 1], f32, tag="rc")
                    nc.vector.reciprocal(out=rec, in_=pt[:, H:H + 1])
                    ot = ot_pool.tile([128, H], f32, tag="o")
                    nc.vector.tensor_scalar(out=ot, in0=pt[:, 0:H], scalar1=rec,
                                            scalar2=None, op0=OP.mult)
                    r0 = qb * QL + j * 128
                    nc.sync.dma_start(out=out[r0:r0 + 128, :], in_=ot)

    nc.compile()
    return nc


def _get_nc():
    if "nc" not in _CACHE:
        _CACHE["nc"] = _build_program()
    return _CACHE["nc"]


def _make_in_maps(x, buffer, mask, Wq, bq, Wk, bk, Wv, bv):
    x = np.asarray(x, dtype=np.float32)
    buffer = np.asarray(buffer, dtype=np.float32)
    mask = np.asarray(mask)
    wqT = np.ascontiguousarray(np.asarray(Wq, np.float32).T).astype(np.float16)
    wkT = np.ascontiguousarray(np.asarray(Wk, np.float32).T).astype(np.float16)
    wvT = np.ascontiguousarray(np.asarray(Wv, np.float32).T).astype(np.float16)
    bq2 = np.ascontiguousarray(np.asarray(bq, np.float32).reshape(H, 1))
    bk2 = np.ascontiguousarray(np.asarray(bk, np.float32).reshape(H, 1))
    bv2 = np.ascontiguousarray(np.asarray(bv, np.float32).reshape(H, 1))
    in_maps = []
    for c in range(8):
        b, half = divmod(c, 2)
        qs = slice(half * Q, (half + 1) * Q)
        maskT_c = np.ascontiguousarray(~(mask[b, qs].astype(bool)).T)
        maskT_c = maskT_c.astype(np.float32).astype(_F8)
        in_maps.append({
            "xT": np.ascontiguousarray(x[b, qs].T).astype(np.float16),
            "bufT": np.ascontiguousarray(buffer[b].T).astype(np.float16),
            "maskT": maskT_c,
            "wqT": wqT, "wkT": wkT, "wvT": wvT,
            "bq2": bq2, "bk2": bk2, "bv2": bv2,
            "onesc": _ONESC, "negid8": _NEGID8,
        })
    return in_maps


def _make_sharded(nc, in_maps):
    """8-device dispatch mirroring bass2jax.run_bass_via_pjrt's multicore path,
    but with inputs device_put + blocked before execute (avoids a first-run
    upload/execute race seen on high-numbered cores)."""
    import jax
    from jax.sharding import Mesh, PartitionSpec, NamedSharding
    from jax.experimental.shard_map import shard_map
    from concourse import bass2jax, mybir
    from concourse.bass2jax import _bass_exec_p, partition_id_tensor

    bass2jax.install_neuronx_cc_hook()
    n_cores = len(in_maps)
    partition_name = nc.partition_id_tensor.name if nc.partition_id_tensor else None
    in_names, out_names, out_avals, zero_shapes = [], [], [], []
    for alloc in nc.m.functions[0].allocations:
        if not isinstance(alloc, mybir.MemoryLocationSet):
            continue
        name = alloc.memorylocations[0].name
        if alloc.kind == "ExternalInput":
            if name != partition_name:
                in_names.append(name)
        elif alloc.kind == "ExternalOutput":
            shape = tuple(alloc.tensor_shape)
            dtype = mybir.dt.np(alloc.dtype)
            out_names.append(name)
            out_avals.append(jax.core.ShapedArray(shape, dtype))
            zero_shapes.append((shape, dtype))
    n_params = len(in_names)
    n_outs = len(out_avals)
    in_names_all = in_names + out_names + ([partition_name] if partition_name else [])
    donate = tuple(range(n_params, n_params + n_outs))

    def _body(*args):
        operands = list(args)
        if partition_name is not None:
            operands.append(partition_id_tensor())
        outs = _bass_exec_p.bind(
            *operands, out_avals=tuple(out_avals), in_names=tuple(in_names_all),
            out_names=tuple(out_names), lowering_input_output_aliases=(),
            sim_require_finite=True, sim_require_nnan=True, nc=nc)
        return tuple(outs)

    devices = jax.devices()[:n_cores]
    mesh = Mesh(np.asarray(devices), ("core",))
    sharded = jax.jit(
        shard_map(_body, mesh=mesh,
                  in_specs=(PartitionSpec("core"),) * (n_params + n_outs),
                  out_specs=(PartitionSpec("core"),) * n_outs,
                  check_rep=False),
        donate_argnums=donate, keep_unused=True)
    sh = NamedSharding(mesh, PartitionSpec("core"))

    concat_in = [np.concatenate([np.asarray(in_maps[c][name]) for c in range(n_cores)],
                                axis=0) for name in in_names]
    dev_in = [jax.device_put(a, sh) for a in concat_in]
    for a in dev_in:
        a.block_until_ready()

    def run_once():
        zeros = [jax.device_put(np.zeros((n_cores * s[0],) + tuple(s[1:]), d), sh)
                 for s, d in zero_shapes]
        for z in zeros:
            z.block_until_ready()
        outs = sharded(*dev_in, *zeros)
        for o in outs:
            o.block_until_ready()
        return [{name: np.asarray(outs[i]).reshape(n_cores, *out_avals[i].shape)[c]
                 for i, name in enumerate(out_names)} for c in range(n_cores)]

    return run_once


def _gather(results):
    out = np.empty((B, QF, H), np.float32)
    for c in range(8):
        b, half = divmod(c, 2)
        out[b, half * Q:(half + 1) * Q] = results[c]["out"]
    return out


def _run(inputs, trace=False):
    nc = _get_nc()
    in_maps = _make_in_maps(**inputs)
    if trace:
        # warm-up execute first (first run after load can race input upload),
        # then a traced execute via run_bass_kernel_spmd for the NTFF profile.
        from concourse.bass_utils import run_bass_kernel_spmd
        run_bass_kernel_spmd(nc, in_maps, list(range(8)), trace=False)
        res = run_bass_kernel_spmd(nc, in_maps, list(range(8)), trace=True)
        return _gather(res.results), res
    run_once = _make_sharded(nc, in_maps)
    run_once()          # warm-up: discard first execution
    results = run_once()
    return _gather(results), None


def kernel(**inputs):
    out, _ = _run(inputs, trace=False)
    return out


# revision 21
# speedup vs baseline: 1.0674x; 1.0064x over previous
"""BufferAttend1d on 8 NeuronCores (TRN2, Bass/Tile).

Sharding: data-parallel over batch (4 batches x 2 cores), sequence-parallel
over Q within a batch (2048 queries/core), full K/V buffer per core.

Per-core device layout ("layout B"): the softmax/contraction k-dim stays on
SBUF partitions throughout:
  queryT [64, 2048], keysT [64, 4096]       (h on partitions)
  vals_ext [4096, 65]                       (k on partitions; col 64 = ones)
  logitsT/eTm tiles [128k, 1024q]           (k on partitions)
Host pre-transposes x, buffer, mask and weights so all device matmuls have
their contraction dim on partitions (no large on-chip transposes), and sends
x/buffer/weights as fp16 (halves DMA, enables full-rate warm matmuls - f32r
keeps the PE HAM clock-gate cold and runs at half clock).
The ones-column in vals makes the PV matmul also produce the softmax
denominator (row 64 of readT); the final 128x65 PE-transpose puts the
denominator on the free axis for a per-partition tensor_scalar divide.
Masking: eTm = exp(logits/8) * mask, multiplied as u8 {0,1}; exact zeros match
the reference's exp(-1024 - max) == 0 in fp32.
"""

import numpy as np
from contextlib import ExitStack

# Per-core shard shapes (hardcoded per problem spec)
B = 4
QF = 4096          # full queries per batch
Q = 2048           # queries per core
K = 4096           # keys
D = 512            # model dim
H = 64             # key/value head dim
QL = 1024          # q block for the logits/exp/mask/PV stage
NQB = Q // QL      # 2
NKT = K // 128     # 32 k-tiles
NDC = D // 128     # 4 d-chunks
SCALE = 1.0 / 8.0  # 1/sqrt(64)

_CACHE = {}
_ONESC = np.ones((128, K // 128, 1), np.float16)
import ml_dtypes as _mld
_F8 = _mld.float8_e4m3
_NEGID8 = (np.eye(128, dtype=np.float32) * -240.0).astype(_F8)


def _build_program():
    import concourse.mybir as mybir
    import concourse.tile as tile
    from concourse import bacc
    from concourse.masks import make_identity

    f32 = mybir.dt.float32
    f16 = mybir.dt.float16
    f8 = mybir.dt.float8e4
    u8 = mybir.dt.uint8
    AF = mybir.ActivationFunctionType
    OP = mybir.AluOpType

    from concourse.tile import add_dep_helper

    nc = bacc.Bacc("TRN2", target_bir_lowering=False, debug=False, num_devices=8)

    pe_prev = [None]

    def pe_chain(inst):
        raw = inst.ins if hasattr(inst, "ins") else inst
        if pe_prev[0] is not None:
            add_dep_helper(raw, pe_prev[0], sync=False,
                           reason="pin PE stream order")
        pe_prev[0] = raw
        return inst

    xT = nc.dram_tensor("xT", [D, Q], f16, kind="ExternalInput").ap()
    bufT = nc.dram_tensor("bufT", [D, K], f16, kind="ExternalInput").ap()
    maskT = nc.dram_tensor("maskT", [K, Q], f8, kind="ExternalInput").ap()
    negid8 = nc.dram_tensor("negid8", [128, 128], f8, kind="ExternalInput").ap()
    wqT = nc.dram_tensor("wqT", [D, H], f16, kind="ExternalInput").ap()
    wkT = nc.dram_tensor("wkT", [D, H], f16, kind="ExternalInput").ap()
    wvT = nc.dram_tensor("wvT", [D, H], f16, kind="ExternalInput").ap()
    bq2 = nc.dram_tensor("bq2", [H, 1], f32, kind="ExternalInput").ap()
    bk2 = nc.dram_tensor("bk2", [H, 1], f32, kind="ExternalInput").ap()
    bv2 = nc.dram_tensor("bv2", [H, 1], f32, kind="ExternalInput").ap()
    onesc = nc.dram_tensor("onesc", [128, NKT, 1], f16, kind="ExternalInput").ap()
    out = nc.dram_tensor("out", [Q, H], f32, kind="ExternalOutput").ap()

    xT_c = xT.rearrange("(c p) q -> c p q", p=128)       # [NDC, 128, Q]
    bufT_c = bufT.rearrange("(c p) k -> c p k", p=128)   # [NDC, 128, K]
    maskT_v = maskT.rearrange("(t p) q -> p t q", p=128)  # [128, NKT, Q]
    wqT_c = wqT.rearrange("(c p) h -> p c h", p=128)     # [128, NDC, H]
    wkT_c = wkT.rearrange("(c p) h -> p c h", p=128)
    wvT_c = wvT.rearrange("(c p) h -> p c h", p=128)

    with tile.TileContext(nc) as tc, ExitStack() as ctx:
        const = ctx.enter_context(tc.tile_pool(name="const", bufs=1))
        persist = ctx.enter_context(tc.tile_pool(name="persist", bufs=1))

        # constants
        wq_sb = const.tile([128, NDC, H], f16, tag="wq")
        wk_sb = const.tile([128, NDC, H], f16, tag="wk")
        wv_sb = const.tile([128, NDC, H], f16, tag="wv")
        nc.sync.dma_start(out=wq_sb, in_=wqT_c)
        nc.sync.dma_start(out=wk_sb, in_=wkT_c)
        nc.sync.dma_start(out=wv_sb, in_=wvT_c)
        bq_sb = const.tile([H, 1], f32, tag="bq")
        bk_sb = const.tile([H, 1], f32, tag="bk")
        bv_sb2 = const.tile([H, 1], f32, tag="bv2")
        nc.sync.dma_start(out=bq_sb, in_=bq2)
        nc.sync.dma_start(out=bk_sb, in_=bk2)
        nc.sync.dma_start(out=bv_sb2, in_=bv2)
        ident = const.tile([128, 128], f32, tag="ident")
        make_identity(nc, ident)
        ident16 = const.tile([128, 128], f16, tag="ident16")
        make_identity(nc, ident16)
        negid8_sb = const.tile([128, 128], f8, tag="negid8")
        nc.sync.dma_start(out=negid8_sb, in_=negid8)

        # persistent activations; queryT/keysT duplicated on partitions 64-127
        # so pairs of QK matmuls can run on distinct PE row-groups concurrently
        queryT = persist.tile([128, Q], f16, tag="queryT")
        keysT = persist.tile([128, K], f16, tag="keysT")
        vals = persist.tile([128, NKT, H + 1], f16, tag="vals")

        # ---- Phase 1: projections ----
        with tc.tile_pool(name="xin", bufs=NDC) as xin_pool, \
             tc.tile_pool(name="bin", bufs=NDC) as bin_pool, \
             tc.tile_pool(name="pp64", bufs=2, space="PSUM") as pp64, \
             tc.tile_pool(name="ppv", bufs=2, space="PSUM") as ppv:
            xt = []
            for dc in range(NDC):
                t = xin_pool.tile([128, Q], f16)
                nc.sync.dma_start(out=t, in_=xT_c[dc])
                xt.append(t)
            bt = []
            for dc in range(NDC):
                t = bin_pool.tile([128, K], f16)
                nc.sync.dma_start(out=t, in_=bufT_c[dc])
                bt.append(t)

            # queryT[h, q] = sum_d WqT[d, h] * xT[d, q]  (+ bq per-partition)
            for qb in range(Q // 512):
                ps = pp64.tile([H, 512], f32, tag="p64")
                qs = slice(qb * 512, (qb + 1) * 512)
                for dc in range(NDC):
                    pe_chain(nc.tensor.matmul(ps, lhsT=wq_sb[:, dc, :],
                                              rhs=xt[dc][:, qs],
                                              start=(dc == 0), stop=(dc == NDC - 1)))
                nc.scalar.activation(out=queryT[0:H, qs], in_=ps, func=AF.Identity,
                                     bias=bq_sb, scale=1.0)
                nc.scalar.activation(out=queryT[H:2 * H, qs], in_=ps,
                                     func=AF.Identity, bias=bq_sb, scale=1.0)
            # keysT[h, k]
            for kb in range(K // 512):
                ps = pp64.tile([H, 512], f32, tag="p64")
                ks = slice(kb * 512, (kb + 1) * 512)
                for dc in range(NDC):
                    pe_chain(nc.tensor.matmul(ps, lhsT=wk_sb[:, dc, :],
                                              rhs=bt[dc][:, ks],
                                              start=(dc == 0), stop=(dc == NDC - 1)))
                nc.scalar.activation(out=keysT[0:H, ks], in_=ps, func=AF.Identity,
                                     bias=bk_sb, scale=1.0)
                nc.scalar.activation(out=keysT[H:2 * H, ks], in_=ps,
                                     func=AF.Identity, bias=bk_sb, scale=1.0)
            # valsT[v, k] (fast N=512 matmuls), add bias via per-partition ACT,
            # then PE-transpose 64x128 tiles into vals[k, v] natural layout.
            valsT = persist.tile([H, K], f16, tag="valsT")
            for kb in range(K // 512):
                ps = pp64.tile([H, 512], f32, tag="p64")
                ks = slice(kb * 512, (kb + 1) * 512)
                for dc in range(NDC):
                    pe_chain(nc.tensor.matmul(ps, lhsT=wv_sb[:, dc, :],
                                              rhs=bt[dc][:, ks],
                                              start=(dc == 0), stop=(dc == NDC - 1)))
                nc.scalar.activation(out=valsT[:, ks], in_=ps, func=AF.Identity,
                                     bias=bv_sb2, scale=1.0)
            for kt in range(NKT):
                pv16 = ppv.tile([128, H], f16, tag="pv")
                pe_chain(nc.tensor.transpose(pv16, valsT[:, kt * 128:(kt + 1) * 128],
                                             ident16[0:H, 0:H]))
                nc.scalar.copy(out=vals[:, kt, 0:H], in_=pv16)
            nc.sync.dma_start(out=vals[:, :, H:H + 1], in_=onesc)

        # ---- Phase 2: attention over q-blocks of QL ----
        rts = []
        with tc.tile_pool(name="mask", bufs=NQB) as mask_pool, \
             tc.tile_pool(name="et", bufs=NKT + 2) as et_pool, \
             tc.tile_pool(name="rt", bufs=NQB) as rt_pool, \
             tc.tile_pool(name="psl", bufs=3, space="PSUM") as psl_pool, \
             tc.tile_pool(name="psr", bufs=1, space="PSUM") as psr_pool:
            mslabs = []
            for qb in range(NQB):
                qs = slice(qb * QL, (qb + 1) * QL)
                mp = mask_pool.tile([128, NKT, QL], f8, tag="m", name=f"mslab{qb}")
                nc.sync.dma_start(out=mp, in_=maskT_v[:, :, qs])
                mslabs.append(mp)
            for qb in range(NQB):
                qs = slice(qb * QL, (qb + 1) * QL)
                mslab = mslabs[qb]
                ets = []
                pr = psr_pool.tile([H + 1, QL], f32, tag="r")

                def emit_pv(kt):
                    for h in range(QL // 512):
                        hs = slice(h * 512, (h + 1) * 512)
                        pe_chain(nc.tensor.matmul(
                            pr[:, hs], lhsT=vals[:, kt, :], rhs=ets[kt][:, hs],
                            start=(kt == 0), stop=(kt == NKT - 1)))

                PV_LAG = 2  # pairs
                for pair in range(NKT // 2):
                    kta, ktb = 2 * pair, 2 * pair + 1
                    pls = {kta: psl_pool.tile([128, QL], f32, tag="l", name=f"pl{qb}_{kta}"),
                           ktb: psl_pool.tile([128, QL], f32, tag="l", name=f"pl{qb}_{ktb}")}
                    for h in range(QL // 512):
                        hs = slice(h * 512, (h + 1) * 512)
                        qhs = slice(qb * QL + h * 512, qb * QL + (h + 1) * 512)
                        # adjacent pair on PE row-groups 0-63 / 64-127
                        pe_chain(nc.tensor.matmul(
                            pls[kta][:, hs],
                            lhsT=keysT[0:H, kta * 128:(kta + 1) * 128],
                            rhs=queryT[0:H, qhs], start=True, stop=False))
                        pe_chain(nc.tensor.matmul(
                            pls[ktb][:, hs],
                            lhsT=keysT[H:2 * H, ktb * 128:(ktb + 1) * 128],
                            rhs=queryT[H:2 * H, qhs], start=True, stop=False))
                        # add -240 to masked-out logits: exp underflows to 0
                        pe_chain(nc.tensor.matmul(
                            pls[kta][:, hs], lhsT=negid8_sb,
                            rhs=mslab[:, kta, hs], start=False, stop=True))
                        pe_chain(nc.tensor.matmul(
                            pls[ktb][:, hs], lhsT=negid8_sb,
                            rhs=mslab[:, ktb, hs], start=False, stop=True))
                    for kt in (kta, ktb):
                        et = et_pool.tile([128, QL], f16, tag="e")
                        nc.scalar.activation(out=et, in_=pls[kt], func=AF.Exp,
                                             scale=SCALE)
                        ets.append(et)
                    if pair >= PV_LAG:
                        for kt in (2 * (pair - PV_LAG), 2 * (pair - PV_LAG) + 1):
                            emit_pv(kt)
                for pair in range(NKT // 2 - PV_LAG, NKT // 2):
                    for kt in (2 * pair, 2 * pair + 1):
                        emit_pv(kt)
                rt = rt_pool.tile([H + 1, QL], f32, tag="rt")
                nc.vector.tensor_copy(out=rt, in_=pr)
                rts.append(rt)

        # ---- Phase 3: transpose readT, divide by denominator, store ----
        with tc.tile_pool(name="ot", bufs=3) as ot_pool, \
             tc.tile_pool(name="rec", bufs=3) as rec_pool, \
             tc.tile_pool(name="pst", bufs=3, space="PSUM") as pst_pool:
            for qb in range(NQB):
                rt = rts[qb]
                for j in range(QL // 128):
                    pt = pst_pool.tile([128, H + 1], f32, tag="t")
                    pe_chain(nc.tensor.transpose(pt, rt[:, j * 128:(j + 1) * 128],
                                                 ident[0:H + 1, 0:H + 1]))
                    rec = rec_pool.tile([128, 1], f32, tag="rc")
                    nc.vector.reciprocal(out=rec, in_=pt[:, H:H + 1])
                    ot = ot_pool.tile([128, H], f32, tag="o")
                    nc.vector.tensor_scalar(out=ot, in0=pt[:, 0:H], scalar1=rec,
                                            scalar2=None, op0=OP.mult)
                    r0 = qb * QL + j * 128
                    nc.sync.dma_start(out=out[r0:r0 + 128, :], in_=ot)

    nc.compile()
    return nc


def _get_nc():
    if "nc" not in _CACHE:
        _CACHE["nc"] = _build_program()
    return _CACHE["nc"]


def _make_in_maps(x, buffer, mask, Wq, bq, Wk, bk, Wv, bv):
    x = np.asarray(x, dtype=np.float32)
    buffer = np.asarray(buffer, dtype=np.float32)
    mask = np.asarray(mask)
    wqT = np.ascontiguousarray(np.asarray(Wq, np.float32).T).astype(np.float16)
    wkT = np.ascontiguousarray(np.asarray(Wk, np.float32).T).astype(np.float16)
    wvT = np.ascontiguousarray(np.asarray(Wv, np.float32).T).astype(np.float16)
    bq2 = np.ascontiguousarray(np.asarray(bq, np.float32).reshape(H, 1))
    bk2 = np.ascontiguousarray(np.asarray(bk, np.float32).reshape(H, 1))
    bv2 = np.ascontiguousarray(np.asarray(bv, np.float32).reshape(H, 1))
    in_maps = []
    for c in range(8):
        b, half = divmod(c, 2)
        qs = slice(half * Q, (half + 1) * Q)
        maskT_c = np.ascontiguousarray(~(mask[b, qs].astype(bool)).T)
        maskT_c = maskT_c.astype(np.float32).astype(_F8)
        in_maps.append({
            "xT": np.ascontiguousarray(x[b, qs].T).astype(np.float16),
            "bufT": np.ascontiguousarray(buffer[b].T).astype(np.float16),
            "maskT": maskT_c,
            "wqT": wqT, "wkT": wkT, "wvT": wvT,
            "bq2": bq2, "bk2": bk2, "bv2": bv2,
            "onesc": _ONESC, "negid8": _NEGID8,
        })
    return in_maps


def _make_sharded(nc, in_maps):
    """8-device dispatch mirroring bass2jax.run_bass_via_pjrt's multicore path,
    but with inputs device_put + blocked before execute (avoids a first-run
    upload/execute race seen on high-numbered cores)."""
    import jax
    from jax.sharding import Mesh, PartitionSpec, NamedSharding
    from jax.experimental.shard_map import shard_map
    from concourse import bass2jax, mybir
    from concourse.bass2jax import _bass_exec_p, partition_id_tensor

    bass2jax.install_neuronx_cc_hook()
    n_cores = len(in_maps)
    partition_name = nc.partition_id_tensor.name if nc.partition_id_tensor else None
    in_names, out_names, out_avals, zero_shapes = [], [], [], []
    for alloc in nc.m.functions[0].allocations:
        if not isinstance(alloc, mybir.MemoryLocationSet):
            continue
        name = alloc.memorylocations[0].name
        if alloc.kind == "ExternalInput":
            if name != partition_name:
                in_names.append(name)
        elif alloc.kind == "ExternalOutput":
            shape = tuple(alloc.tensor_shape)
            dtype = mybir.dt.np(alloc.dtype)
            out_names.append(name)
            out_avals.append(jax.core.ShapedArray(shape, dtype))
            zero_shapes.append((shape, dtype))
    n_params = len(in_names)
    n_outs = len(out_avals)
    in_names_all = in_names + out_names + ([partition_name] if partition_name else [])
    donate = tuple(range(n_params, n_params + n_outs))

    def _body(*args):
        operands = list(args)
        if partition_name is not None:
            operands.append(partition_id_tensor())
        outs = _bass_exec_p.bind(
            *operands, out_avals=tuple(out_avals), in_names=tuple(in_names_all),
            out_names=tuple(out_names), lowering_input_output_aliases=(),
            sim_require_finite=True, sim_require_nnan=True, nc=nc)
        return tuple(outs)

    devices = jax.devices()[:n_cores]
    mesh = Mesh(np.asarray(devices), ("core",))
    sharded = jax.jit(
        shard_map(_body, mesh=mesh,
                  in_specs=(PartitionSpec("core"),) * (n_params + n_outs),
                  out_specs=(PartitionSpec("core"),) * n_outs,
                  check_rep=False),
        donate_argnums=donate, keep_unused=True)
    sh = NamedSharding(mesh, PartitionSpec("core"))

    concat_in = [np.concatenate([np.asarray(in_maps[c][name]) for c in range(n_cores)],
                                axis=0) for name in in_names]
    dev_in = [jax.device_put(a, sh) for a in concat_in]
    for a in dev_in:
        a.block_until_ready()

    def run_once():
        zeros = [jax.device_put(np.zeros((n_cores * s[0],) + tuple(s[1:]), d), sh)
                 for s, d in zero_shapes]
        for z in zeros:
            z.block_until_ready()
        outs = sharded(*dev_in, *zeros)
        for o in outs:
            o.block_until_ready()
        return [{name: np.asarray(outs[i]).reshape(n_cores, *out_avals[i].shape)[c]
                 for i, name in enumerate(out_names)} for c in range(n_cores)]

    return run_once


def _gather(results):
    out = np.empty((B, QF, H), np.float32)
    for c in range(8):
        b, half = divmod(c, 2)
        out[b, half * Q:(half + 1) * Q] = results[c]["out"]
    return out


def _run(inputs, trace=False):
    nc = _get_nc()
    in_maps = _make_in_maps(**inputs)
    if trace:
        # warm-up execute first (first run after load can race input upload),
        # then a traced execute via run_bass_kernel_spmd for the NTFF profile.
        from concourse.bass_utils import run_bass_kernel_spmd
        run_bass_kernel_spmd(nc, in_maps, list(range(8)), trace=False)
        res = run_bass_kernel_spmd(nc, in_maps, list(range(8)), trace=True)
        return _gather(res.results), res
    run_once = _make_sharded(nc, in_maps)
    run_once()          # warm-up: discard first execution
    results = run_once()
    return _gather(results), None


def kernel(**inputs):
    out, _ = _run(inputs, trace=False)
    return out
